# revision 13
# baseline (speedup 1.0000x reference)
"""MoE LoRA linear kernel for Trainium2, 8 NeuronCores, data-parallel over tokens.

Reference computation (per token x, D=4096, E=28 experts, rank 8, top-2):
  base   = x @ W^T
  logits = x @ gate_W^T ; top-2 softmax -> per-expert gates g (0 elsewhere)
  h_e    = x @ A_e^T                     (all experts, rank 8)
  out    = base + sum_e g_e*2 * h_e @ B_e^T

Sharding: tokens split 8 ways (1024 tokens/core); weights replicated.

Numerics: all matmul operands bf16 with fp32 PSUM accumulation
(end-to-end rel err ~2.7e-3 incl. the handful of top-2 flips from bf16
gate logits).  bf16 enables the PE's automatic Fast Weight Load (~51ns
vs ~78ns fp32r per 128-col stationary load; loads between consecutive
matmuls sharing the stationary are skipped by HW) and halves all DMA.

Structure (per core):
  - x arrives host-packed+cast as xt [P, KT*T] bf16 (k-major, resident;
    first 4-kt slab DMA'd separately so compute starts early).
  - [lora_A (224 rows) | gate_W (28) | pad] is one stationary block 'at'
    (Act-queue DMA, parallel with xt): 32 accumulating matmuls x 2
    stationaries x 2 token chunks produce h'^T [224, T] AND gate
    logits^T [28, T] in 4 PSUM banks, already in the er-major layout the
    rank-combine needs (no h transposes).
  - logits^T -> partition-shift DMA -> 8 small PE transposes ->
    token-major top-2 softmax chain (DVE + Act) -> per-tile rank expand
    -> 16 PE transposes -> mskT [224, T]; h'T *= mskT in place.  All of
    it hides inside base GEMM group 0's k-loop via 3 hooks.
  - base GEMM: 11 groups of (<=3 j-tiles x 2 token chunks); ONE PSUM
    pool of 8 rotating bank-slots serves h' accumulators, transposes and
    group psums (no static split -> groups overlap their drain).  Per
    group ONE 3MB W DMA ([P, KT*384] bf16, 24KB/partition contiguous
    lines, double-buffered; group 0 on the Act queue), 32x6 accumulating
    matmuls ordered so each stationary serves both token chunks
    back-to-back, then 4 rank-combine matmuls per j (ba pair then bb
    pair), Act-engine copy-out, out DMA on the Act queue.

Measured (8-core SPMD, marginal body time via reps-slope): ~612us vs
928us baseline; CoreSim cost model says 510us with PE 96.8% busy -- the
gap is unmodeled LDWEIGHTS serialization (~51ns x ~1240 distinct loads)
plus ~13ns/matmul instruction overhead.
"""
import sys

for _p in ("/opt/trn_rl_repo", "/root/.axon_site/_ro/trn_rl_repo"):
    if _p not in sys.path:
        sys.path.insert(0, _p)

import ml_dtypes
import numpy as np

import concourse.bass as bass
import concourse.mybir as mybir
import concourse.tile as tile
from concourse import bacc, bass_utils
from concourse.masks import make_identity

F32 = mybir.dt.float32
BF16 = mybir.dt.bfloat16
BF = ml_dtypes.bfloat16

N_CORES = 8
B, S, D_IN, D_OUT = 4, 2048, 4096, 4096
NE, RANK, SCALING = 28, 8, 2.0
ER = NE * RANK                 # 224
T = B * S // N_CORES           # 1024 tokens per core
P = 128
KT = D_IN // P                 # 32 k-tiles
JT = D_OUT // P                # 32 output row-tiles
TT = T // P                    # 8 token tiles
NCH = 512                      # token chunk (one PSUM bank)
TCH = T // NCH                 # 2 token chunks
JG = 3                         # j-tiles per psum group (3x2 chunks = 6 banks)
NG = (JT + JG - 1) // JG       # 11 groups (last has 2 j-tiles)
AT_COLS = 256                  # 224 lora + 28 gate + 4 zero pad
GROUPS = [(g * JG, min(JG, JT - g * JG)) for g in range(NG)]


def build_nc(reps=1, gating=True):
    """reps>1 replicates the whole body inside one NEFF (timing only: the
    launch-train slope between two reps values isolates one body's device
    time, cancelling per-launch dispatch overhead)."""
    nc = bacc.Bacc("TRN2", target_bir_lowering=False, debug=False)
    xt_d = nc.dram_tensor("xt", [P, KT * T], BF16, kind="ExternalInput").ap()
    wt_d = nc.dram_tensor("wt", [NG, P, KT, JG * P], BF16,
                          kind="ExternalInput").ap()
    at_d = nc.dram_tensor("at", [P, KT * AT_COLS], BF16,
                          kind="ExternalInput").ap()
    ba_d = nc.dram_tensor("ba", [P, D_OUT], BF16, kind="ExternalInput").ap()
    bb_d = nc.dram_tensor("bb", [ER - P, D_OUT], BF16,
                          kind="ExternalInput").ap()
    out_d = nc.dram_tensor("out", [D_OUT, T], F32, kind="ExternalOutput").ap()

    xt_re = xt_d.rearrange("p (kt t) -> p kt t", kt=KT)
    at_re = at_d.rearrange("p (kt c) -> p kt c", kt=KT)

    with tile.TileContext(nc) as tc:
        with (
            tc.tile_pool(name="resident", bufs=1) as rp,
            tc.tile_pool(name="wstream", bufs=2) as wp,
            tc.tile_pool(name="outstage", bufs=4) as op_,
            tc.tile_pool(name="gating", bufs=1) as gp,
            tc.tile_pool(name="smalls", bufs=2) as sp,
            tc.tile_pool(name="psmm", bufs=8, space="PSUM") as psm,
        ):
            for _rep in range(reps):
                _body(nc, rp, wp, op_, gp, sp, psm, psm,
                      xt_re, wt_d, at_re, ba_d, bb_d, out_d, gating)
    nc.compile()
    return nc


def _body(nc, rp, wp, op_, gp, sp, pht, psm,
          xt_re, wt_d, at_re, ba_d, bb_d, out_d, gating=True):
    xt_sb = rp.tile([P, KT, T], BF16)
    nc.sync.dma_start(xt_sb[:, 0:4], xt_re[:, 0:4])
    nc.sync.dma_start(xt_sb[:, 4:8], xt_re[:, 4:8])
    # at on the Act DMA queue so h' can start after ~max(xt q0, at);
    # first slab separate so h' kt=0 isn't gated on the whole tensor
    at_sb = rp.tile([P, KT, AT_COLS], BF16)
    nc.scalar.dma_start(at_sb[:, 0:4], at_re[:, 0:4])
    nc.scalar.dma_start(at_sb[:, 4:], at_re[:, 4:])
    ba_sb = rp.tile([P, D_OUT], BF16)
    nc.scalar.dma_start(ba_sb[:], ba_d[:])
    bb_sb = rp.tile([ER - P, D_OUT], BF16)
    nc.scalar.dma_start(bb_sb[:], bb_d[:])
    identf = rp.tile([P, P], F32)
    make_identity(nc, identf[:])
    identb = rp.tile([P, P], BF16)
    nc.vector.tensor_copy(identb[:], identf[:])
    for q in range(1, 4):
        nc.sync.dma_start(xt_sb[:, q * 8:(q + 1) * 8],
                          xt_re[:, q * 8:(q + 1) * 8])

    w_tiles = {}

    def load_w(g, eng=None):
        j0, nj = GROUPS[g]
        w_t = wp.tile([P, KT, JG * P], BF16, name="w_t")
        (eng or nc.sync).dma_start(w_t[:, :, :nj * P], wt_d[g, :, :, :nj * P])
        w_tiles[g] = w_t

    # group 0's W on the Act queue: SP is busy with xt for ~25us
    load_w(0, eng=nc.scalar)

    hraw_a = rp.tile([P, T], BF16)
    hraw_b = rp.tile([ER - P, T], BF16)   # er 128..223
    lgtmp = rp.tile([P, T], F32)          # psum copy parks on partitions 96:124
    lg_sb = rp.tile([NE, T], F32)         # logits^T shifted to partitions 0:28
    logits_all = rp.tile([P, TT, NE], F32)
    gsc_all = rp.tile([P, TT, NE], F32)
    mskT_a = rp.tile([P, T], BF16)
    mskT_b = rp.tile([ER - P, T], BF16)

    # ---- h'^T + logits^T: at-stationary accumulating matmuls ----
    hp = {(s, c): psm.tile([P, NCH], F32, name=f"hp{s}{c}", tag="pm")
          for s in (0, 1) for c in range(TCH)}
    for kt in range(KT):
        for s in (0, 1):
            lhsT = at_sb[:, kt, s * P:(s + 1) * P]
            for c in range(TCH):
                nc.tensor.matmul(hp[s, c], lhsT,
                                 xt_sb[:, kt, c * NCH:(c + 1) * NCH],
                                 start=(kt == 0), stop=(kt == KT - 1))
    for c in range(TCH):
        cs = slice(c * NCH, (c + 1) * NCH)
        nc.vector.tensor_copy(hraw_a[:, cs], hp[0, c][:])
        nc.vector.tensor_copy(hraw_b[:, cs], hp[1, c][0:ER - P])
        nc.vector.tensor_copy(lgtmp[96:96 + NE, cs], hp[1, c][96:96 + NE])
    # partition shift 96:124 -> 0:28 (engines can't shift; SBUF->SBUF DMA can)
    nc.scalar.dma_start(lg_sb[:], lgtmp[96:96 + NE, :])

    def lg_transposes():
        for t in range(TT):
            plg = pht.tile([P, NE], F32, name="plg", tag="pm")
            nc.tensor.transpose(plg[:], lg_sb[:, t * P:(t + 1) * P],
                                identf[0:NE, 0:NE])
            nc.vector.tensor_copy(logits_all[:, t], plg[:])

    def gate_chain():
        """Batched top-2 softmax for all 8 token tiles."""
        n = TT
        EB = (P, n, NE)
        m1 = gp.tile([P, n], F32, name="m1", tag="m1")
        nc.vector.reduce_max(m1[:], logits_all[:], axis=mybir.AxisListType.X)
        m1b = m1[:, :, None].to_broadcast(EB)
        eq = gp.tile([P, n, NE], F32, name="eq", tag="eq")
        nc.vector.tensor_tensor(eq[:], logits_all[:], m1b,
                                mybir.AluOpType.is_equal)
        nc.vector.scalar_tensor_tensor(
            eq[:], eq[:], -1e30, logits_all[:],
            mybir.AluOpType.mult, mybir.AluOpType.add)
        m2 = gp.tile([P, n], F32, name="m2", tag="m2")
        nc.vector.reduce_max(m2[:], eq[:], axis=mybir.AxisListType.X)
        mask2 = gp.tile([P, n, NE], F32, name="mask2", tag="mask2")
        nc.vector.tensor_tensor(mask2[:], logits_all[:],
                                m2[:, :, None].to_broadcast(EB),
                                mybir.AluOpType.is_ge)
        d1 = gp.tile([P, n, NE], F32, name="d1", tag="d1")
        nc.vector.tensor_tensor(d1[:], logits_all[:], m1b,
                                mybir.AluOpType.subtract)
        nc.scalar.activation(d1[:], d1[:], mybir.ActivationFunctionType.Exp)
        d2 = gp.tile([P, n], F32, name="d2", tag="d2")
        nc.vector.tensor_tensor(d2[:], m2[:], m1[:],
                                mybir.AluOpType.subtract)
        nc.scalar.activation(d2[:], d2[:], mybir.ActivationFunctionType.Exp)
        nc.vector.tensor_scalar_add(d2[:], d2[:], 1.0)
        nc.vector.reciprocal(d2[:], d2[:])
        nc.vector.tensor_scalar_mul(d2[:], d2[:], SCALING)
        nc.vector.tensor_tensor(d1[:], d1[:], mask2[:],
                                mybir.AluOpType.mult)
        nc.vector.tensor_tensor(gsc_all[:], d1[:],
                                d2[:, :, None].to_broadcast(EB),
                                mybir.AluOpType.mult)

    def msk_tile(t):
        ts_ = slice(t * P, (t + 1) * P)
        msk = sp.tile([P, ER], BF16, name="msk")
        nc.vector.tensor_copy(
            msk[:].rearrange("p (e r) -> p e r", r=RANK),
            gsc_all[:, t, :, None].to_broadcast((P, NE, RANK)))
        pta = pht.tile([P, P], BF16, name="pta", tag="pm")
        nc.tensor.transpose(pta[:], msk[:, 0:P], identb[:])
        nc.vector.tensor_copy(mskT_a[:, ts_], pta[:])
        ptb = pht.tile([ER - P, P], BF16, name="ptb", tag="pm")
        nc.tensor.transpose(ptb[:], msk[:, P:ER], identb[:])
        nc.vector.tensor_copy(mskT_b[:, ts_], ptb[:])

    def apply_mask():
        nc.vector.tensor_tensor(hraw_a[:], hraw_a[:], mskT_a[:],
                                mybir.AluOpType.mult)
        nc.vector.tensor_tensor(hraw_b[:], hraw_b[:], mskT_b[:],
                                mybir.AluOpType.mult)

    hooks = {
        2: lambda: (lg_transposes(), gate_chain()),
        12: lambda: [msk_tile(t) for t in range(TT)],
        18: apply_mask,
    } if gating else {}

    # ---- base GEMM + rank combine ----
    for g, (j0, nj) in enumerate(GROUPS):
        w_t = w_tiles.pop(g)
        psums = {(j, c): psm.tile([P, NCH], F32, name=f"pm{j}{c}", tag="pm")
                 for j in range(nj) for c in range(TCH)}
        for kt in range(KT):
            if kt == 2 and g + 1 < NG:
                load_w(g + 1)
            for j in range(nj):
                lhsT = w_t[:, kt, j * P:(j + 1) * P]
                for c in range(TCH):
                    nc.tensor.matmul(psums[j, c], lhsT,
                                     xt_sb[:, kt, c * NCH:(c + 1) * NCH],
                                     start=(kt == 0), stop=False)
            if g == 0 and kt in hooks:
                hooks[kt]()
        for j in range(nj):
            ja = slice((j0 + j) * P, (j0 + j + 1) * P)
            for c in range(TCH):
                cs = slice(c * NCH, (c + 1) * NCH)
                nc.tensor.matmul(psums[j, c], ba_sb[:, ja], hraw_a[:, cs],
                                 start=False, stop=False)
            for c in range(TCH):
                cs = slice(c * NCH, (c + 1) * NCH)
                nc.tensor.matmul(psums[j, c], bb_sb[:, ja], hraw_b[:, cs],
                                 start=False, stop=True)
                ot = op_.tile([P, NCH], F32, name="ot")
                nc.scalar.activation(ot[:], psums[j, c][:],
                                     mybir.ActivationFunctionType.Copy)
                nc.scalar.dma_start(out_d[ja, cs], ot[:])


_NC_CACHE = None
_LAST_IN_MAPS = None


def _get_nc():
    global _NC_CACHE
    if _NC_CACHE is None:
        _NC_CACHE = build_nc()
    return _NC_CACHE


def kernel(x, base_W, gate_W, lora_A, lora_B):
    x = np.asarray(x, dtype=np.float32)
    base_W = np.asarray(base_W, dtype=np.float32)
    gate_W = np.asarray(gate_W, dtype=np.float32)
    lora_A = np.asarray(lora_A, dtype=np.float32)
    lora_B = np.asarray(lora_B, dtype=np.float32)

    xf = x.reshape(B * S, D_IN)
    # W: [NG, P, KT, JG*P], wt[g,p,kt,c] = W[g*JG*P + c, kt*P + p]
    wpad = np.zeros((NG * JG * P, D_IN), dtype=BF)
    wpad[:D_OUT] = base_W.astype(BF)
    wt_np = np.ascontiguousarray(
        wpad.reshape(NG, JG * P, KT, P).transpose(0, 3, 2, 1))
    # at: [lora_A flat (224) | gate_W (28) | pad(4)] -> [P, KT*AT_COLS]
    M = np.zeros((AT_COLS, D_IN), np.float32)
    M[:ER] = lora_A.reshape(ER, D_IN)
    M[ER:ER + NE] = gate_W
    at_np = np.ascontiguousarray(
        M.T.reshape(KT, P, AT_COLS).transpose(1, 0, 2).reshape(
            P, KT * AT_COLS)).astype(BF)
    # lora_B -> b_flat [(e r), D_out], halves at er=128
    b_flat = lora_B.transpose(0, 2, 1).reshape(ER, D_OUT)

    ba_np = np.ascontiguousarray(b_flat[:P]).astype(BF)
    bb_np = np.ascontiguousarray(b_flat[P:]).astype(BF)

    in_maps = []
    for c in range(N_CORES):
        xc = xf[c * T:(c + 1) * T]                       # [T, D_in]
        xt_np = np.ascontiguousarray(
            xc.T.reshape(KT, P, T).transpose(1, 0, 2).reshape(
                P, KT * T)).astype(BF)
        in_maps.append({
            "xt": xt_np,
            "wt": wt_np,
            "at": at_np,
            "ba": ba_np,
            "bb": bb_np,
        })

    global _LAST_IN_MAPS
    _LAST_IN_MAPS = in_maps
    nc = _get_nc()
    res = bass_utils.run_bass_kernel_spmd(nc, in_maps,
                                          core_ids=list(range(N_CORES)))
    out = np.empty((B * S, D_OUT), dtype=np.float32)
    for c in range(N_CORES):
        out[c * T:(c + 1) * T] = res.results[c]["out"].T
    return out.reshape(B, S, D_OUT)


# revision 17
# speedup vs baseline: 1.0233x; 1.0233x over previous
"""MoE LoRA linear kernel for Trainium2, 8 NeuronCores, data-parallel over tokens.

Reference computation (per token x, D=4096, E=28 experts, rank 8, top-2):
  base   = x @ W^T
  logits = x @ gate_W^T ; top-2 softmax -> per-expert gates g (0 elsewhere)
  h_e    = x @ A_e^T                     (all experts, rank 8)
  out    = base + sum_e g_e*2 * h_e @ B_e^T

Sharding: tokens split 8 ways (1024 tokens/core); weights replicated.

Numerics: all matmul operands bf16 with fp32 PSUM accumulation
(end-to-end rel err ~2.7e-3 incl. the handful of top-2 flips from bf16
gate logits).  bf16 enables the PE's automatic Fast Weight Load (~51ns
vs ~78ns fp32r per 128-col stationary load; loads between consecutive
matmuls sharing the stationary are skipped by HW) and halves all DMA.

Structure (per core):
  - x arrives host-packed+cast as xt [P, KT*T] bf16 (k-major, resident;
    first 4-kt slab DMA'd separately so compute starts early).
  - [lora_A (224 rows) | gate_W (28) | pad] is one stationary block 'at'
    (Act-queue DMA, parallel with xt): 32 accumulating matmuls x 2
    stationaries x 2 token chunks produce h'^T [224, T] AND gate
    logits^T [28, T] in 4 PSUM banks, already in the er-major layout the
    rank-combine needs (no h transposes).
  - logits^T -> partition-shift DMA -> 8 small PE transposes ->
    token-major top-2 softmax chain (DVE + Act) -> per-tile rank expand
    -> 16 PE transposes -> mskT [224, T]; h'T *= mskT in place.  All of
    it hides inside base GEMM group 0's k-loop via 3 hooks.
  - base GEMM: 11 groups of (<=3 j-tiles x 2 token chunks); ONE PSUM
    pool of 8 rotating bank-slots serves h' accumulators, transposes and
    group psums (no static split -> groups overlap their drain).  Per
    group ONE 3MB W DMA ([P, KT*384] bf16, 24KB/partition contiguous
    lines, double-buffered; group 0 on the Act queue), 32x6 accumulating
    matmuls ordered so each stationary serves both token chunks
    back-to-back, then 4 rank-combine matmuls per j (ba pair then bb
    pair), Act-engine copy-out, out DMA on the Act queue.

Measured (8-core SPMD, marginal body time via reps-slope): ~612us vs
928us baseline; CoreSim cost model says 510us with PE 96.8% busy -- the
gap is unmodeled LDWEIGHTS serialization (~51ns x ~1240 distinct loads)
plus ~13ns/matmul instruction overhead.
"""
import sys

for _p in ("/opt/trn_rl_repo", "/root/.axon_site/_ro/trn_rl_repo"):
    if _p not in sys.path:
        sys.path.insert(0, _p)

import ml_dtypes
import numpy as np

import concourse.bass as bass
import concourse.mybir as mybir
import concourse.tile as tile
from concourse import bacc, bass_utils
from concourse.masks import make_identity

F32 = mybir.dt.float32
BF16 = mybir.dt.bfloat16
BF = ml_dtypes.bfloat16

N_CORES = 8
B, S, D_IN, D_OUT = 4, 2048, 4096, 4096
NE, RANK, SCALING = 28, 8, 2.0
ER = NE * RANK                 # 224
T = B * S // N_CORES           # 1024 tokens per core
P = 128
KT = D_IN // P                 # 32 k-tiles
JT = D_OUT // P                # 32 output row-tiles
TT = T // P                    # 8 token tiles
NCH = 512                      # token chunk (one PSUM bank)
TCH = T // NCH                 # 2 token chunks
JG = 3                         # j-tiles per psum group (3x2 chunks = 6 banks)
NG = (JT + JG - 1) // JG       # 11 groups (last has 2 j-tiles)
AT_COLS = 256                  # 224 lora + 28 gate + 4 zero pad
GROUPS = [(g * JG, min(JG, JT - g * JG)) for g in range(NG)]


def build_nc(reps=1, gating=True):
    """reps>1 replicates the whole body inside one NEFF (timing only: the
    launch-train slope between two reps values isolates one body's device
    time, cancelling per-launch dispatch overhead)."""
    nc = bacc.Bacc("TRN2", target_bir_lowering=False, debug=False)
    xt_d = nc.dram_tensor("xt", [P, KT * T], BF16, kind="ExternalInput").ap()
    wt_d = nc.dram_tensor("wt", [NG, P, KT, JG * P], BF16,
                          kind="ExternalInput").ap()
    at_d = nc.dram_tensor("at", [P, KT * AT_COLS], BF16,
                          kind="ExternalInput").ap()
    ba_d = nc.dram_tensor("ba", [P, D_OUT], BF16, kind="ExternalInput").ap()
    bb_d = nc.dram_tensor("bb", [ER - P, D_OUT], BF16,
                          kind="ExternalInput").ap()
    out_d = nc.dram_tensor("out", [D_OUT, T], F32, kind="ExternalOutput").ap()

    xt_re = xt_d.rearrange("p (kt t) -> p kt t", kt=KT)
    at_re = at_d.rearrange("p (kt c) -> p kt c", kt=KT)

    with tile.TileContext(nc) as tc:
        with (
            tc.tile_pool(name="resident", bufs=1) as rp,
            tc.tile_pool(name="wstream", bufs=2) as wp,
            tc.tile_pool(name="outstage", bufs=4) as op_,
            tc.tile_pool(name="gating", bufs=1) as gp,
            tc.tile_pool(name="smalls", bufs=2) as sp,
            tc.tile_pool(name="psmm", bufs=8, space="PSUM") as psm,
        ):
            for _rep in range(reps):
                _body(nc, rp, wp, op_, gp, sp, psm, psm,
                      xt_re, wt_d, at_re, ba_d, bb_d, out_d, gating)
    nc.compile()
    return nc


def _body(nc, rp, wp, op_, gp, sp, pht, psm,
          xt_re, wt_d, at_re, ba_d, bb_d, out_d, gating=True):
    xt_sb = rp.tile([P, KT, T], BF16)
    nc.sync.dma_start(xt_sb[:, 0:4], xt_re[:, 0:4])
    nc.sync.dma_start(xt_sb[:, 4:8], xt_re[:, 4:8])
    # at on the Act DMA queue so h' can start after ~max(xt q0, at);
    # first slab separate so h' kt=0 isn't gated on the whole tensor
    at_sb = rp.tile([P, KT, AT_COLS], BF16)
    nc.scalar.dma_start(at_sb[:, 0:4], at_re[:, 0:4])
    nc.scalar.dma_start(at_sb[:, 4:], at_re[:, 4:])
    identf = rp.tile([P, P], F32)
    make_identity(nc, identf[:])
    identb = rp.tile([P, P], BF16)
    nc.vector.tensor_copy(identb[:], identf[:])
    for q in range(1, 4):
        nc.sync.dma_start(xt_sb[:, q * 8:(q + 1) * 8],
                          xt_re[:, q * 8:(q + 1) * 8])
    # B weights aren't needed until the first combine (~80us in): SP tail
    ba_sb = rp.tile([P, D_OUT], BF16)
    nc.sync.dma_start(ba_sb[:], ba_d[:])
    bb_sb = rp.tile([ER - P, D_OUT], BF16)
    nc.sync.dma_start(bb_sb[:], bb_d[:])

    w_tiles = {}

    def load_w(g, eng=None):
        j0, nj = GROUPS[g]
        w_t = wp.tile([P, KT, JG * P], BF16, name="w_t")
        (eng or nc.sync).dma_start(w_t[:, :, :nj * P], wt_d[g, :, :, :nj * P])
        w_tiles[g] = w_t

    # group 0's W on the Act queue: SP is busy with xt for ~25us
    load_w(0, eng=nc.scalar)

    hraw_a = rp.tile([P, T], BF16)
    hraw_b = rp.tile([ER - P, T], BF16)   # er 128..223
    lgtmp = rp.tile([P, T], F32)          # psum copy parks on partitions 96:124
    lg_sb = rp.tile([NE, T], F32)         # logits^T shifted to partitions 0:28
    logits_all = rp.tile([P, TT, NE], F32)
    gsc_all = rp.tile([P, TT, NE], F32)
    mskT_a = rp.tile([P, T], BF16)
    mskT_b = rp.tile([ER - P, T], BF16)

    # ---- h'^T + logits^T: at-stationary accumulating matmuls ----
    hp = {(s, c): psm.tile([P, NCH], F32, name=f"hp{s}{c}", tag="pm")
          for s in (0, 1) for c in range(TCH)}
    for kt in range(KT):
        for s in (0, 1):
            lhsT = at_sb[:, kt, s * P:(s + 1) * P]
            for c in range(TCH):
                nc.tensor.matmul(hp[s, c], lhsT,
                                 xt_sb[:, kt, c * NCH:(c + 1) * NCH],
                                 start=(kt == 0), stop=(kt == KT - 1))
    for c in range(TCH):
        cs = slice(c * NCH, (c + 1) * NCH)
        nc.vector.tensor_copy(hraw_a[:, cs], hp[0, c][:])
        nc.vector.tensor_copy(hraw_b[:, cs], hp[1, c][0:ER - P])
        nc.vector.tensor_copy(lgtmp[96:96 + NE, cs], hp[1, c][96:96 + NE])
    # partition shift 96:124 -> 0:28 (engines can't shift; SBUF->SBUF DMA can)
    nc.scalar.dma_start(lg_sb[:], lgtmp[96:96 + NE, :])

    def lg_transposes():
        for t in range(TT):
            plg = pht.tile([P, NE], F32, name="plg", tag="pm")
            nc.tensor.transpose(plg[:], lg_sb[:, t * P:(t + 1) * P],
                                identf[0:NE, 0:NE])
            nc.vector.tensor_copy(logits_all[:, t], plg[:])

    def gate_chain():
        """Batched top-2 softmax for all 8 token tiles."""
        n = TT
        EB = (P, n, NE)
        m1 = gp.tile([P, n], F32, name="m1", tag="m1")
        nc.vector.reduce_max(m1[:], logits_all[:], axis=mybir.AxisListType.X)
        m1b = m1[:, :, None].to_broadcast(EB)
        eq = gp.tile([P, n, NE], F32, name="eq", tag="eq")
        nc.vector.tensor_tensor(eq[:], logits_all[:], m1b,
                                mybir.AluOpType.is_equal)
        nc.vector.scalar_tensor_tensor(
            eq[:], eq[:], -1e30, logits_all[:],
            mybir.AluOpType.mult, mybir.AluOpType.add)
        m2 = gp.tile([P, n], F32, name="m2", tag="m2")
        nc.vector.reduce_max(m2[:], eq[:], axis=mybir.AxisListType.X)
        mask2 = gp.tile([P, n, NE], F32, name="mask2", tag="mask2")
        nc.vector.tensor_tensor(mask2[:], logits_all[:],
                                m2[:, :, None].to_broadcast(EB),
                                mybir.AluOpType.is_ge)
        d1 = gp.tile([P, n, NE], F32, name="d1", tag="d1")
        nc.vector.tensor_tensor(d1[:], logits_all[:], m1b,
                                mybir.AluOpType.subtract)
        nc.scalar.activation(d1[:], d1[:], mybir.ActivationFunctionType.Exp)
        d2 = gp.tile([P, n], F32, name="d2", tag="d2")
        nc.vector.tensor_tensor(d2[:], m2[:], m1[:],
                                mybir.AluOpType.subtract)
        nc.scalar.activation(d2[:], d2[:], mybir.ActivationFunctionType.Exp)
        nc.vector.tensor_scalar_add(d2[:], d2[:], 1.0)
        nc.vector.reciprocal(d2[:], d2[:])
        nc.vector.tensor_scalar_mul(d2[:], d2[:], SCALING)
        nc.vector.tensor_tensor(d1[:], d1[:], mask2[:],
                                mybir.AluOpType.mult)
        nc.vector.tensor_tensor(gsc_all[:], d1[:],
                                d2[:, :, None].to_broadcast(EB),
                                mybir.AluOpType.mult)

    def msk_tile(t):
        ts_ = slice(t * P, (t + 1) * P)
        msk = sp.tile([P, ER], BF16, name="msk")
        nc.vector.tensor_copy(
            msk[:].rearrange("p (e r) -> p e r", r=RANK),
            gsc_all[:, t, :, None].to_broadcast((P, NE, RANK)))
        pta = pht.tile([P, P], BF16, name="pta", tag="pm")
        nc.tensor.transpose(pta[:], msk[:, 0:P], identb[:])
        nc.vector.tensor_copy(mskT_a[:, ts_], pta[:])
        ptb = pht.tile([ER - P, P], BF16, name="ptb", tag="pm")
        nc.tensor.transpose(ptb[:], msk[:, P:ER], identb[:])
        nc.vector.tensor_copy(mskT_b[:, ts_], ptb[:])

    def apply_mask():
        nc.vector.tensor_tensor(hraw_a[:], hraw_a[:], mskT_a[:],
                                mybir.AluOpType.mult)
        nc.vector.tensor_tensor(hraw_b[:], hraw_b[:], mskT_b[:],
                                mybir.AluOpType.mult)

    hooks = {
        2: lambda: (lg_transposes(), gate_chain()),
        12: lambda: [msk_tile(t) for t in range(TT)],
        18: apply_mask,
    } if gating else {}

    # ---- base GEMM + rank combine ----
    for g, (j0, nj) in enumerate(GROUPS):
        w_t = w_tiles.pop(g)
        psums = {(j, c): psm.tile([P, NCH], F32, name=f"pm{j}{c}", tag="pm")
                 for j in range(nj) for c in range(TCH)}
        for kt in range(KT):
            if kt == 2 and g + 1 < NG:
                load_w(g + 1)
            for j in range(nj):
                lhsT = w_t[:, kt, j * P:(j + 1) * P]
                for c in range(TCH):
                    nc.tensor.matmul(psums[j, c], lhsT,
                                     xt_sb[:, kt, c * NCH:(c + 1) * NCH],
                                     start=(kt == 0), stop=False)
            if g == 0 and kt in hooks:
                hooks[kt]()
        for j in range(nj):
            ja = slice((j0 + j) * P, (j0 + j + 1) * P)
            for c in range(TCH):
                cs = slice(c * NCH, (c + 1) * NCH)
                nc.tensor.matmul(psums[j, c], ba_sb[:, ja], hraw_a[:, cs],
                                 start=False, stop=False)
            for c in range(TCH):
                cs = slice(c * NCH, (c + 1) * NCH)
                nc.tensor.matmul(psums[j, c], bb_sb[:, ja], hraw_b[:, cs],
                                 start=False, stop=True)
                ot = op_.tile([P, NCH], F32, name="ot")
                if g == NG - 1 and c == 1:
                    # tail drain off the critical path: split the last
                    # group's copy-out across DVE and both DMA queues
                    nc.vector.tensor_copy(ot[:], psums[j, c][:])
                    nc.sync.dma_start(out_d[ja, cs], ot[:])
                else:
                    nc.scalar.activation(ot[:], psums[j, c][:],
                                         mybir.ActivationFunctionType.Copy)
                    nc.scalar.dma_start(out_d[ja, cs], ot[:])


_NC_CACHE = None
_LAST_IN_MAPS = None


def _get_nc():
    global _NC_CACHE
    if _NC_CACHE is None:
        _NC_CACHE = build_nc()
    return _NC_CACHE


def kernel(x, base_W, gate_W, lora_A, lora_B):
    x = np.asarray(x, dtype=np.float32)
    base_W = np.asarray(base_W, dtype=np.float32)
    gate_W = np.asarray(gate_W, dtype=np.float32)
    lora_A = np.asarray(lora_A, dtype=np.float32)
    lora_B = np.asarray(lora_B, dtype=np.float32)

    xf = x.reshape(B * S, D_IN)
    # W: [NG, P, KT, JG*P], wt[g,p,kt,c] = W[g*JG*P + c, kt*P + p]
    wpad = np.zeros((NG * JG * P, D_IN), dtype=BF)
    wpad[:D_OUT] = base_W.astype(BF)
    wt_np = np.ascontiguousarray(
        wpad.reshape(NG, JG * P, KT, P).transpose(0, 3, 2, 1))
    # at: [lora_A flat (224) | gate_W (28) | pad(4)] -> [P, KT*AT_COLS]
    M = np.zeros((AT_COLS, D_IN), np.float32)
    M[:ER] = lora_A.reshape(ER, D_IN)
    M[ER:ER + NE] = gate_W
    at_np = np.ascontiguousarray(
        M.T.reshape(KT, P, AT_COLS).transpose(1, 0, 2).reshape(
            P, KT * AT_COLS)).astype(BF)
    # lora_B -> b_flat [(e r), D_out], halves at er=128
    b_flat = lora_B.transpose(0, 2, 1).reshape(ER, D_OUT)

    ba_np = np.ascontiguousarray(b_flat[:P]).astype(BF)
    bb_np = np.ascontiguousarray(b_flat[P:]).astype(BF)

    in_maps = []
    for c in range(N_CORES):
        xc = xf[c * T:(c + 1) * T]                       # [T, D_in]
        xt_np = np.ascontiguousarray(
            xc.T.reshape(KT, P, T).transpose(1, 0, 2).reshape(
                P, KT * T)).astype(BF)
        in_maps.append({
            "xt": xt_np,
            "wt": wt_np,
            "at": at_np,
            "ba": ba_np,
            "bb": bb_np,
        })

    global _LAST_IN_MAPS
    _LAST_IN_MAPS = in_maps
    nc = _get_nc()
    res = bass_utils.run_bass_kernel_spmd(nc, in_maps,
                                          core_ids=list(range(N_CORES)))
    out = np.empty((B * S, D_OUT), dtype=np.float32)
    for c in range(N_CORES):
        out[c * T:(c + 1) * T] = res.results[c]["out"].T
    return out.reshape(B, S, D_OUT)


# revision 19
# speedup vs baseline: 1.0299x; 1.0064x over previous
"""MoE LoRA linear kernel for Trainium2, 8 NeuronCores, data-parallel over tokens.

Reference computation (per token x, D=4096, E=28 experts, rank 8, top-2):
  base   = x @ W^T
  logits = x @ gate_W^T ; top-2 softmax -> per-expert gates g (0 elsewhere)
  h_e    = x @ A_e^T                     (all experts, rank 8)
  out    = base + sum_e g_e*2 * h_e @ B_e^T

Sharding: tokens split 8 ways (1024 tokens/core); weights replicated.

Numerics: all matmul operands bf16 with fp32 PSUM accumulation
(end-to-end rel err ~2.7e-3 incl. the handful of top-2 flips from bf16
gate logits).  bf16 enables the PE's automatic Fast Weight Load (~51ns
vs ~78ns fp32r per 128-col stationary load; loads between consecutive
matmuls sharing the stationary are skipped by HW) and halves all DMA.

Structure (per core):
  - x arrives host-packed+cast as xt [P, KT*T] bf16 (k-major, resident;
    first 4-kt slab DMA'd separately so compute starts early).
  - [lora_A (224 rows) | gate_W (28) | pad] is one stationary block 'at'
    (Act-queue DMA, parallel with xt): 32 accumulating matmuls x 2
    stationaries x 2 token chunks produce h'^T [224, T] AND gate
    logits^T [28, T] in 4 PSUM banks, already in the er-major layout the
    rank-combine needs (no h transposes).
  - logits^T -> partition-shift DMA -> 8 small PE transposes ->
    token-major top-2 softmax chain (DVE + Act) -> per-tile rank expand
    -> 16 PE transposes -> mskT [224, T]; h'T *= mskT in place.  All of
    it hides inside base GEMM group 0's k-loop via 3 hooks.
  - base GEMM: 11 groups of (<=3 j-tiles x 2 token chunks); ONE PSUM
    pool of 8 rotating bank-slots serves h' accumulators, transposes and
    group psums (no static split -> groups overlap their drain).  Per
    group ONE 3MB W DMA ([P, KT*384] bf16, 24KB/partition contiguous
    lines, double-buffered; group 0 on the Act queue), 32x6 accumulating
    matmuls ordered so each stationary serves both token chunks
    back-to-back, then 4 rank-combine matmuls per j (ba pair then bb
    pair), Act-engine copy-out, out DMA on the Act queue.

Measured (8-core SPMD, marginal body time via reps-slope): ~575-610us
vs 928us baseline (whose true dispatch-free body is ~706us); CoreSim
cost model says 510us with PE 96.8% busy -- the remaining gap is
unmodeled LDWEIGHTS serialization (~51ns x ~1240 distinct stationary
loads) plus ~13ns/matmul instruction overhead.  The last group's
copy-out is split across DVE+Act engines and both DMA queues so the
tail drain stays off the critical path.
"""
import sys

for _p in ("/opt/trn_rl_repo", "/root/.axon_site/_ro/trn_rl_repo"):
    if _p not in sys.path:
        sys.path.insert(0, _p)

import ml_dtypes
import numpy as np

import concourse.bass as bass
import concourse.mybir as mybir
import concourse.tile as tile
from concourse import bacc, bass_utils
from concourse.masks import make_identity

F32 = mybir.dt.float32
BF16 = mybir.dt.bfloat16
BF = ml_dtypes.bfloat16

N_CORES = 8
B, S, D_IN, D_OUT = 4, 2048, 4096, 4096
NE, RANK, SCALING = 28, 8, 2.0
ER = NE * RANK                 # 224
T = B * S // N_CORES           # 1024 tokens per core
P = 128
KT = D_IN // P                 # 32 k-tiles
JT = D_OUT // P                # 32 output row-tiles
TT = T // P                    # 8 token tiles
NCH = 512                      # token chunk (one PSUM bank)
TCH = T // NCH                 # 2 token chunks
JG = 3                         # j-tiles per psum group (3x2 chunks = 6 banks)
NG = (JT + JG - 1) // JG       # 11 groups (last has 2 j-tiles)
AT_COLS = 256                  # 224 lora + 28 gate + 4 zero pad
GROUPS = [(g * JG, min(JG, JT - g * JG)) for g in range(NG)]


def build_nc(reps=1, gating=True):
    """reps>1 replicates the whole body inside one NEFF (timing only: the
    launch-train slope between two reps values isolates one body's device
    time, cancelling per-launch dispatch overhead)."""
    nc = bacc.Bacc("TRN2", target_bir_lowering=False, debug=False)
    xt_d = nc.dram_tensor("xt", [P, KT * T], BF16, kind="ExternalInput").ap()
    wt_d = nc.dram_tensor("wt", [NG, P, KT, JG * P], BF16,
                          kind="ExternalInput").ap()
    at_d = nc.dram_tensor("at", [P, KT * AT_COLS], BF16,
                          kind="ExternalInput").ap()
    ba_d = nc.dram_tensor("ba", [P, D_OUT], BF16, kind="ExternalInput").ap()
    bb_d = nc.dram_tensor("bb", [ER - P, D_OUT], BF16,
                          kind="ExternalInput").ap()
    out_d = nc.dram_tensor("out", [D_OUT, T], F32, kind="ExternalOutput").ap()

    xt_re = xt_d.rearrange("p (kt t) -> p kt t", kt=KT)
    at_re = at_d.rearrange("p (kt c) -> p kt c", kt=KT)

    with tile.TileContext(nc) as tc:
        with (
            tc.tile_pool(name="resident", bufs=1) as rp,
            tc.tile_pool(name="wstream", bufs=2) as wp,
            tc.tile_pool(name="outstage", bufs=4) as op_,
            tc.tile_pool(name="gating", bufs=1) as gp,
            tc.tile_pool(name="smalls", bufs=2) as sp,
            tc.tile_pool(name="psmm", bufs=8, space="PSUM") as psm,
        ):
            for _rep in range(reps):
                _body(nc, rp, wp, op_, gp, sp, psm, psm,
                      xt_re, wt_d, at_re, ba_d, bb_d, out_d, gating)
    nc.compile()
    return nc


def _body(nc, rp, wp, op_, gp, sp, pht, psm,
          xt_re, wt_d, at_re, ba_d, bb_d, out_d, gating=True):
    xt_sb = rp.tile([P, KT, T], BF16)
    for a, b in ((0, 4), (4, 8), (8, 12), (12, 16)):
        nc.sync.dma_start(xt_sb[:, a:b], xt_re[:, a:b])
    # at on the Act DMA queue so h' can start after ~max(xt q0, at);
    # first slab separate so h' kt=0 isn't gated on the whole tensor
    at_sb = rp.tile([P, KT, AT_COLS], BF16)
    nc.scalar.dma_start(at_sb[:, 0:4], at_re[:, 0:4])
    nc.scalar.dma_start(at_sb[:, 4:], at_re[:, 4:])
    identf = rp.tile([P, P], F32)
    make_identity(nc, identf[:])
    identb = rp.tile([P, P], BF16)
    nc.vector.tensor_copy(identb[:], identf[:])
    for q in range(2, 4):
        nc.sync.dma_start(xt_sb[:, q * 8:(q + 1) * 8],
                          xt_re[:, q * 8:(q + 1) * 8])
    # B weights aren't needed until the first combine (~80us in): SP tail
    ba_sb = rp.tile([P, D_OUT], BF16)
    nc.sync.dma_start(ba_sb[:], ba_d[:])
    bb_sb = rp.tile([ER - P, D_OUT], BF16)
    nc.sync.dma_start(bb_sb[:], bb_d[:])

    w_tiles = {}

    def load_w(g, eng=None):
        j0, nj = GROUPS[g]
        w_t = wp.tile([P, KT, JG * P], BF16, name="w_t")
        (eng or nc.sync).dma_start(w_t[:, :, :nj * P], wt_d[g, :, :, :nj * P])
        w_tiles[g] = w_t

    # group 0's W on the Act queue: SP is busy with xt for ~25us
    load_w(0, eng=nc.scalar)

    hraw_a = rp.tile([P, T], BF16)
    hraw_b = rp.tile([ER - P, T], BF16)   # er 128..223
    lgtmp = rp.tile([P, T], F32)          # psum copy parks on partitions 96:124
    lg_sb = rp.tile([NE, T], F32)         # logits^T shifted to partitions 0:28
    logits_all = rp.tile([P, TT, NE], F32)
    gsc_all = rp.tile([P, TT, NE], F32)
    mskT_a = rp.tile([P, T], BF16)
    mskT_b = rp.tile([ER - P, T], BF16)

    # ---- h'^T + logits^T: at-stationary accumulating matmuls ----
    hp = {(s, c): psm.tile([P, NCH], F32, name=f"hp{s}{c}", tag="pm")
          for s in (0, 1) for c in range(TCH)}
    for kt in range(KT):
        for s in (0, 1):
            lhsT = at_sb[:, kt, s * P:(s + 1) * P]
            for c in range(TCH):
                nc.tensor.matmul(hp[s, c], lhsT,
                                 xt_sb[:, kt, c * NCH:(c + 1) * NCH],
                                 start=(kt == 0), stop=(kt == KT - 1))
    for c in range(TCH):
        cs = slice(c * NCH, (c + 1) * NCH)
        nc.vector.tensor_copy(hraw_a[:, cs], hp[0, c][:])
        nc.vector.tensor_copy(hraw_b[:, cs], hp[1, c][0:ER - P])
        nc.vector.tensor_copy(lgtmp[96:96 + NE, cs], hp[1, c][96:96 + NE])
    # partition shift 96:124 -> 0:28 (engines can't shift; SBUF->SBUF DMA can)
    nc.scalar.dma_start(lg_sb[:], lgtmp[96:96 + NE, :])

    def lg_transposes():
        for t in range(TT):
            plg = pht.tile([P, NE], F32, name="plg", tag="pm")
            nc.tensor.transpose(plg[:], lg_sb[:, t * P:(t + 1) * P],
                                identf[0:NE, 0:NE])
            nc.vector.tensor_copy(logits_all[:, t], plg[:])

    def gate_chain():
        """Batched top-2 softmax for all 8 token tiles."""
        n = TT
        EB = (P, n, NE)
        m1 = gp.tile([P, n], F32, name="m1", tag="m1")
        nc.vector.reduce_max(m1[:], logits_all[:], axis=mybir.AxisListType.X)
        m1b = m1[:, :, None].to_broadcast(EB)
        eq = gp.tile([P, n, NE], F32, name="eq", tag="eq")
        nc.vector.tensor_tensor(eq[:], logits_all[:], m1b,
                                mybir.AluOpType.is_equal)
        nc.vector.scalar_tensor_tensor(
            eq[:], eq[:], -1e30, logits_all[:],
            mybir.AluOpType.mult, mybir.AluOpType.add)
        m2 = gp.tile([P, n], F32, name="m2", tag="m2")
        nc.vector.reduce_max(m2[:], eq[:], axis=mybir.AxisListType.X)
        mask2 = gp.tile([P, n, NE], F32, name="mask2", tag="mask2")
        nc.vector.tensor_tensor(mask2[:], logits_all[:],
                                m2[:, :, None].to_broadcast(EB),
                                mybir.AluOpType.is_ge)
        d1 = gp.tile([P, n, NE], F32, name="d1", tag="d1")
        nc.vector.tensor_tensor(d1[:], logits_all[:], m1b,
                                mybir.AluOpType.subtract)
        nc.scalar.activation(d1[:], d1[:], mybir.ActivationFunctionType.Exp)
        d2 = gp.tile([P, n], F32, name="d2", tag="d2")
        nc.vector.tensor_tensor(d2[:], m2[:], m1[:],
                                mybir.AluOpType.subtract)
        nc.scalar.activation(d2[:], d2[:], mybir.ActivationFunctionType.Exp)
        nc.vector.tensor_scalar_add(d2[:], d2[:], 1.0)
        nc.vector.reciprocal(d2[:], d2[:])
        nc.vector.tensor_scalar_mul(d2[:], d2[:], SCALING)
        nc.vector.tensor_tensor(d1[:], d1[:], mask2[:],
                                mybir.AluOpType.mult)
        nc.vector.tensor_tensor(gsc_all[:], d1[:],
                                d2[:, :, None].to_broadcast(EB),
                                mybir.AluOpType.mult)

    def msk_tile(t):
        ts_ = slice(t * P, (t + 1) * P)
        msk = sp.tile([P, ER], BF16, name="msk")
        nc.vector.tensor_copy(
            msk[:].rearrange("p (e r) -> p e r", r=RANK),
            gsc_all[:, t, :, None].to_broadcast((P, NE, RANK)))
        pta = pht.tile([P, P], BF16, name="pta", tag="pm")
        nc.tensor.transpose(pta[:], msk[:, 0:P], identb[:])
        nc.vector.tensor_copy(mskT_a[:, ts_], pta[:])
        ptb = pht.tile([ER - P, P], BF16, name="ptb", tag="pm")
        nc.tensor.transpose(ptb[:], msk[:, P:ER], identb[:])
        nc.vector.tensor_copy(mskT_b[:, ts_], ptb[:])

    def apply_mask():
        nc.vector.tensor_tensor(hraw_a[:], hraw_a[:], mskT_a[:],
                                mybir.AluOpType.mult)
        nc.vector.tensor_tensor(hraw_b[:], hraw_b[:], mskT_b[:],
                                mybir.AluOpType.mult)

    hooks = {
        2: lambda: (lg_transposes(), gate_chain()),
        12: lambda: [msk_tile(t) for t in range(TT)],
        18: apply_mask,
    } if gating else {}

    # ---- base GEMM + rank combine ----
    for g, (j0, nj) in enumerate(GROUPS):
        w_t = w_tiles.pop(g)
        psums = {(j, c): psm.tile([P, NCH], F32, name=f"pm{j}{c}", tag="pm")
                 for j in range(nj) for c in range(TCH)}
        for kt in range(KT):
            if kt == 2 and g + 1 < NG:
                load_w(g + 1)
            for j in range(nj):
                lhsT = w_t[:, kt, j * P:(j + 1) * P]
                for c in range(TCH):
                    nc.tensor.matmul(psums[j, c], lhsT,
                                     xt_sb[:, kt, c * NCH:(c + 1) * NCH],
                                     start=(kt == 0), stop=False)
            if g == 0 and kt in hooks:
                hooks[kt]()
        for j in range(nj):
            ja = slice((j0 + j) * P, (j0 + j + 1) * P)
            for c in range(TCH):
                cs = slice(c * NCH, (c + 1) * NCH)
                nc.tensor.matmul(psums[j, c], ba_sb[:, ja], hraw_a[:, cs],
                                 start=False, stop=False)
            for c in range(TCH):
                cs = slice(c * NCH, (c + 1) * NCH)
                nc.tensor.matmul(psums[j, c], bb_sb[:, ja], hraw_b[:, cs],
                                 start=False, stop=True)
                ot = op_.tile([P, NCH], F32, name="ot")
                if g == NG - 1 and c == 1:
                    # tail drain off the critical path: split the last
                    # group's copy-out across DVE and both DMA queues
                    nc.vector.tensor_copy(ot[:], psums[j, c][:])
                    nc.sync.dma_start(out_d[ja, cs], ot[:])
                else:
                    nc.scalar.activation(ot[:], psums[j, c][:],
                                         mybir.ActivationFunctionType.Copy)
                    nc.scalar.dma_start(out_d[ja, cs], ot[:])


_NC_CACHE = None
_LAST_IN_MAPS = None


def _get_nc():
    global _NC_CACHE
    if _NC_CACHE is None:
        _NC_CACHE = build_nc()
    return _NC_CACHE


def kernel(x, base_W, gate_W, lora_A, lora_B):
    x = np.asarray(x, dtype=np.float32)
    base_W = np.asarray(base_W, dtype=np.float32)
    gate_W = np.asarray(gate_W, dtype=np.float32)
    lora_A = np.asarray(lora_A, dtype=np.float32)
    lora_B = np.asarray(lora_B, dtype=np.float32)

    xf = x.reshape(B * S, D_IN)
    # W: [NG, P, KT, JG*P], wt[g,p,kt,c] = W[g*JG*P + c, kt*P + p]
    wpad = np.zeros((NG * JG * P, D_IN), dtype=BF)
    wpad[:D_OUT] = base_W.astype(BF)
    wt_np = np.ascontiguousarray(
        wpad.reshape(NG, JG * P, KT, P).transpose(0, 3, 2, 1))
    # at: [lora_A flat (224) | gate_W (28) | pad(4)] -> [P, KT*AT_COLS]
    M = np.zeros((AT_COLS, D_IN), np.float32)
    M[:ER] = lora_A.reshape(ER, D_IN)
    M[ER:ER + NE] = gate_W
    at_np = np.ascontiguousarray(
        M.T.reshape(KT, P, AT_COLS).transpose(1, 0, 2).reshape(
            P, KT * AT_COLS)).astype(BF)
    # lora_B -> b_flat [(e r), D_out], halves at er=128
    b_flat = lora_B.transpose(0, 2, 1).reshape(ER, D_OUT)

    ba_np = np.ascontiguousarray(b_flat[:P]).astype(BF)
    bb_np = np.ascontiguousarray(b_flat[P:]).astype(BF)

    in_maps = []
    for c in range(N_CORES):
        xc = xf[c * T:(c + 1) * T]                       # [T, D_in]
        xt_np = np.ascontiguousarray(
            xc.T.reshape(KT, P, T).transpose(1, 0, 2).reshape(
                P, KT * T)).astype(BF)
        in_maps.append({
            "xt": xt_np,
            "wt": wt_np,
            "at": at_np,
            "ba": ba_np,
            "bb": bb_np,
        })

    global _LAST_IN_MAPS
    _LAST_IN_MAPS = in_maps
    nc = _get_nc()
    res = bass_utils.run_bass_kernel_spmd(nc, in_maps,
                                          core_ids=list(range(N_CORES)))
    out = np.empty((B * S, D_OUT), dtype=np.float32)
    for c in range(N_CORES):
        out[c * T:(c + 1) * T] = res.results[c]["out"].T
    return out.reshape(B, S, D_OUT)


# revision 21
# speedup vs baseline: 1.0352x; 1.0051x over previous
"""MoE LoRA linear kernel for Trainium2, 8 NeuronCores, data-parallel over tokens.

Reference computation (per token x, D=4096, E=28 experts, rank 8, top-2):
  base   = x @ W^T
  logits = x @ gate_W^T ; top-2 softmax -> per-expert gates g (0 elsewhere)
  h_e    = x @ A_e^T                     (all experts, rank 8)
  out    = base + sum_e g_e*2 * h_e @ B_e^T

Sharding: tokens split 8 ways (1024 tokens/core); weights replicated.

Numerics: all matmul operands bf16 with fp32 PSUM accumulation
(end-to-end rel err ~2.7e-3 incl. the handful of top-2 flips from bf16
gate logits).  bf16 enables the PE's automatic Fast Weight Load (~51ns
vs ~78ns fp32r per 128-col stationary load; loads between consecutive
matmuls sharing the stationary are skipped by HW) and halves all DMA.

Structure (per core):
  - x arrives host-packed+cast as xt [P, KT*T] bf16 (k-major, resident;
    first 4-kt slab DMA'd separately so compute starts early).
  - [lora_A (224 rows) | gate_W (28) | pad] is one stationary block 'at'
    (Act-queue DMA, parallel with xt): 32 accumulating matmuls x 2
    stationaries x 2 token chunks produce h'^T [224, T] AND gate
    logits^T [28, T] in 4 PSUM banks, already in the er-major layout the
    rank-combine needs (no h transposes).
  - logits^T -> partition-shift DMA -> 8 small PE transposes ->
    token-major top-2 softmax chain (DVE + Act) -> per-tile rank expand
    -> 16 PE transposes -> mskT [224, T]; h'T *= mskT in place.  All of
    it hides inside base GEMM group 0's k-loop via 3 hooks.
  - base GEMM: 11 groups of (<=3 j-tiles x 2 token chunks); ONE PSUM
    pool of 8 rotating bank-slots serves h' accumulators, transposes and
    group psums (no static split -> groups overlap their drain).  Per
    group ONE 3MB W DMA ([P, KT*384] bf16, 24KB/partition contiguous
    lines, double-buffered; group 0 on the Act queue), 32x6 accumulating
    matmuls ordered so each stationary serves both token chunks
    back-to-back, then 4 rank-combine matmuls per j (ba pair then bb
    pair), Act-engine copy-out, out DMA on the Act queue.

Measured (8-core SPMD, marginal body time via reps-slope): ~575-610us
vs 928us baseline (whose true dispatch-free body is ~706us); CoreSim
cost model says 510us with PE 96.8% busy -- the remaining gap is
unmodeled LDWEIGHTS serialization (~51ns x ~1240 distinct stationary
loads) plus ~13ns/matmul instruction overhead.  The last group's
copy-out is split across DVE+Act engines and both DMA queues so the
tail drain stays off the critical path.
"""
import sys

for _p in ("/opt/trn_rl_repo", "/root/.axon_site/_ro/trn_rl_repo"):
    if _p not in sys.path:
        sys.path.insert(0, _p)

import ml_dtypes
import numpy as np

import concourse.bass as bass
import concourse.mybir as mybir
import concourse.tile as tile
from concourse import bacc, bass_utils
from concourse.masks import make_identity

F32 = mybir.dt.float32
BF16 = mybir.dt.bfloat16
BF = ml_dtypes.bfloat16

N_CORES = 8
B, S, D_IN, D_OUT = 4, 2048, 4096, 4096
NE, RANK, SCALING = 28, 8, 2.0
ER = NE * RANK                 # 224
T = B * S // N_CORES           # 1024 tokens per core
P = 128
KT = D_IN // P                 # 32 k-tiles
JT = D_OUT // P                # 32 output row-tiles
TT = T // P                    # 8 token tiles
NCH = 512                      # token chunk (one PSUM bank)
TCH = T // NCH                 # 2 token chunks
JG = 3                         # j-tiles per psum group (3x2 chunks = 6 banks)
NG = (JT + JG - 1) // JG       # 11 groups (last has 2 j-tiles)
AT_COLS = 256                  # 224 lora + 28 gate + 4 zero pad
GROUPS = [(g * JG, min(JG, JT - g * JG)) for g in range(NG)]


def build_nc(reps=1, gating=True):
    """reps>1 replicates the whole body inside one NEFF (timing only: the
    launch-train slope between two reps values isolates one body's device
    time, cancelling per-launch dispatch overhead)."""
    nc = bacc.Bacc("TRN2", target_bir_lowering=False, debug=False)
    xt_d = nc.dram_tensor("xt", [P, KT * T], BF16, kind="ExternalInput").ap()
    wt_d = nc.dram_tensor("wt", [NG, P, KT, JG * P], BF16,
                          kind="ExternalInput").ap()
    at_d = nc.dram_tensor("at", [P, KT * AT_COLS], BF16,
                          kind="ExternalInput").ap()
    ba_d = nc.dram_tensor("ba", [P, D_OUT], BF16, kind="ExternalInput").ap()
    bb_d = nc.dram_tensor("bb", [ER - P, D_OUT], BF16,
                          kind="ExternalInput").ap()
    out_d = nc.dram_tensor("out", [D_OUT, T], F32, kind="ExternalOutput").ap()

    xt_re = xt_d.rearrange("p (kt t) -> p kt t", kt=KT)
    at_re = at_d.rearrange("p (kt c) -> p kt c", kt=KT)

    with tile.TileContext(nc) as tc:
        with (
            tc.tile_pool(name="resident", bufs=1) as rp,
            tc.tile_pool(name="wstream", bufs=2) as wp,
            tc.tile_pool(name="outstage", bufs=4) as op_,
            tc.tile_pool(name="gating", bufs=1) as gp,
            tc.tile_pool(name="smalls", bufs=2) as sp,
            tc.tile_pool(name="psmm", bufs=8, space="PSUM") as psm,
        ):
            for _rep in range(reps):
                _body(nc, rp, wp, op_, gp, sp, psm, psm,
                      xt_re, wt_d, at_re, ba_d, bb_d, out_d, gating)
    nc.compile()
    return nc


def _body(nc, rp, wp, op_, gp, sp, pht, psm,
          xt_re, wt_d, at_re, ba_d, bb_d, out_d, gating=True):
    xt_sb = rp.tile([P, KT, T], BF16)
    for a, b in ((0, 4), (4, 8), (8, 12), (12, 16)):
        nc.sync.dma_start(xt_sb[:, a:b], xt_re[:, a:b])
    # at on the Act DMA queue so h' can start after ~max(xt q0, at);
    # first slab separate so h' kt=0 isn't gated on the whole tensor
    at_sb = rp.tile([P, KT, AT_COLS], BF16)
    nc.scalar.dma_start(at_sb[:, 0:4], at_re[:, 0:4])
    nc.scalar.dma_start(at_sb[:, 4:], at_re[:, 4:])
    identf = rp.tile([P, P], F32)
    make_identity(nc, identf[:])
    identb = rp.tile([P, P], BF16)
    nc.vector.tensor_copy(identb[:], identf[:])
    for q in range(2, 4):
        nc.sync.dma_start(xt_sb[:, q * 8:(q + 1) * 8],
                          xt_re[:, q * 8:(q + 1) * 8])
    # B weights aren't needed until the first combine (~80us in): SP tail
    ba_sb = rp.tile([P, D_OUT], BF16)
    nc.sync.dma_start(ba_sb[:], ba_d[:])
    bb_sb = rp.tile([ER - P, D_OUT], BF16)
    nc.sync.dma_start(bb_sb[:], bb_d[:])

    w_tiles = {}

    def load_w(g, eng=None):
        j0, nj = GROUPS[g]
        w_t = wp.tile([P, KT, JG * P], BF16, name="w_t")
        (eng or nc.sync).dma_start(w_t[:, :, :nj * P], wt_d[g, :, :, :nj * P])
        w_tiles[g] = w_t

    # group 0's W on the Act queue: SP is busy with xt for ~25us
    load_w(0, eng=nc.scalar)

    hraw_a = rp.tile([P, T], BF16)
    hraw_b = rp.tile([ER - P, T], BF16)   # er 128..223
    lgtmp = rp.tile([P, T], F32)          # psum copy parks on partitions 96:124
    lg_sb = rp.tile([NE, T], F32)         # logits^T shifted to partitions 0:28
    logits_all = rp.tile([P, TT, NE], F32)
    gsc_all = rp.tile([P, TT, NE], F32)
    mskT_a = rp.tile([P, T], BF16)
    mskT_b = rp.tile([ER - P, T], BF16)

    # ---- h'^T + logits^T: at-stationary accumulating matmuls ----
    hp = {(s, c): psm.tile([P, NCH], F32, name=f"hp{s}{c}", tag="pm")
          for s in (0, 1) for c in range(TCH)}
    for kt in range(KT):
        for s in (0, 1):
            lhsT = at_sb[:, kt, s * P:(s + 1) * P]
            for c in range(TCH):
                nc.tensor.matmul(hp[s, c], lhsT,
                                 xt_sb[:, kt, c * NCH:(c + 1) * NCH],
                                 start=(kt == 0), stop=(kt == KT - 1))
    for c in range(TCH):
        cs = slice(c * NCH, (c + 1) * NCH)
        nc.vector.tensor_copy(hraw_a[:, cs], hp[0, c][:])
        nc.vector.tensor_copy(hraw_b[:, cs], hp[1, c][0:ER - P])
        nc.vector.tensor_copy(lgtmp[96:96 + NE, cs], hp[1, c][96:96 + NE])
    # partition shift 96:124 -> 0:28 (engines can't shift; SBUF->SBUF DMA can)
    nc.scalar.dma_start(lg_sb[:], lgtmp[96:96 + NE, :])

    def lg_transposes():
        for t in range(TT):
            plg = pht.tile([P, NE], F32, name="plg", tag="pm")
            nc.tensor.transpose(plg[:], lg_sb[:, t * P:(t + 1) * P],
                                identf[0:NE, 0:NE])
            nc.vector.tensor_copy(logits_all[:, t], plg[:])

    def gate_chain():
        """Batched top-2 softmax for all 8 token tiles."""
        n = TT
        EB = (P, n, NE)
        m1 = gp.tile([P, n], F32, name="m1", tag="m1")
        nc.vector.reduce_max(m1[:], logits_all[:], axis=mybir.AxisListType.X)
        m1b = m1[:, :, None].to_broadcast(EB)
        eq = gp.tile([P, n, NE], F32, name="eq", tag="eq")
        nc.vector.tensor_tensor(eq[:], logits_all[:], m1b,
                                mybir.AluOpType.is_equal)
        nc.vector.scalar_tensor_tensor(
            eq[:], eq[:], -1e30, logits_all[:],
            mybir.AluOpType.mult, mybir.AluOpType.add)
        m2 = gp.tile([P, n], F32, name="m2", tag="m2")
        nc.vector.reduce_max(m2[:], eq[:], axis=mybir.AxisListType.X)
        mask2 = gp.tile([P, n, NE], F32, name="mask2", tag="mask2")
        nc.vector.tensor_tensor(mask2[:], logits_all[:],
                                m2[:, :, None].to_broadcast(EB),
                                mybir.AluOpType.is_ge)
        d1 = gp.tile([P, n, NE], F32, name="d1", tag="d1")
        nc.vector.tensor_tensor(d1[:], logits_all[:], m1b,
                                mybir.AluOpType.subtract)
        nc.scalar.activation(d1[:], d1[:], mybir.ActivationFunctionType.Exp)
        d2 = gp.tile([P, n], F32, name="d2", tag="d2")
        nc.vector.tensor_tensor(d2[:], m2[:], m1[:],
                                mybir.AluOpType.subtract)
        nc.scalar.activation(d2[:], d2[:], mybir.ActivationFunctionType.Exp)
        nc.vector.tensor_scalar_add(d2[:], d2[:], 1.0)
        nc.vector.reciprocal(d2[:], d2[:])
        nc.vector.tensor_scalar_mul(d2[:], d2[:], SCALING)
        nc.vector.tensor_tensor(d1[:], d1[:], mask2[:],
                                mybir.AluOpType.mult)
        nc.vector.tensor_tensor(gsc_all[:], d1[:],
                                d2[:, :, None].to_broadcast(EB),
                                mybir.AluOpType.mult)

    def msk_tile(t):
        ts_ = slice(t * P, (t + 1) * P)
        msk = sp.tile([P, ER], BF16, name="msk")
        nc.vector.tensor_copy(
            msk[:].rearrange("p (e r) -> p e r", r=RANK),
            gsc_all[:, t, :, None].to_broadcast((P, NE, RANK)))
        pta = pht.tile([P, P], BF16, name="pta", tag="pm")
        nc.tensor.transpose(pta[:], msk[:, 0:P], identb[:])
        nc.vector.tensor_copy(mskT_a[:, ts_], pta[:])
        ptb = pht.tile([ER - P, P], BF16, name="ptb", tag="pm")
        nc.tensor.transpose(ptb[:], msk[:, P:ER], identb[:])
        nc.vector.tensor_copy(mskT_b[:, ts_], ptb[:])

    def apply_mask():
        nc.vector.tensor_tensor(hraw_a[:], hraw_a[:], mskT_a[:],
                                mybir.AluOpType.mult)
        nc.vector.tensor_tensor(hraw_b[:], hraw_b[:], mskT_b[:],
                                mybir.AluOpType.mult)

    hooks = {
        2: lambda: (lg_transposes(), gate_chain()),
        12: lambda: [msk_tile(t) for t in range(TT)],
        18: apply_mask,
    } if gating else {}

    # ---- base GEMM + rank combine ----
    for g, (j0, nj) in enumerate(GROUPS):
        w_t = w_tiles.pop(g)
        psums = {(j, c): psm.tile([P, NCH], F32, name=f"pm{j}{c}", tag="pm")
                 for j in range(nj) for c in range(TCH)}
        for kt in range(KT):
            if kt == 2 and g + 1 < NG:
                load_w(g + 1)
            for j in range(nj):
                lhsT = w_t[:, kt, j * P:(j + 1) * P]
                for c in range(TCH):
                    nc.tensor.matmul(psums[j, c], lhsT,
                                     xt_sb[:, kt, c * NCH:(c + 1) * NCH],
                                     start=(kt == 0), stop=False)
            if g == 0 and kt in hooks:
                hooks[kt]()
        for j in range(nj):
            ja = slice((j0 + j) * P, (j0 + j + 1) * P)
            for c in range(TCH):
                cs = slice(c * NCH, (c + 1) * NCH)
                nc.tensor.matmul(psums[j, c], ba_sb[:, ja], hraw_a[:, cs],
                                 start=False, stop=False)
            for c in range(TCH):
                cs = slice(c * NCH, (c + 1) * NCH)
                nc.tensor.matmul(psums[j, c], bb_sb[:, ja], hraw_b[:, cs],
                                 start=False, stop=True)
                ot = op_.tile([P, NCH], F32, name="ot")
                if g == NG - 1 and c == 1:
                    # tail drain off the critical path: split the last
                    # group's copy-out across DVE and both DMA queues
                    nc.vector.tensor_copy(ot[:], psums[j, c][:])
                    nc.sync.dma_start(out_d[ja, cs], ot[:])
                else:
                    nc.scalar.activation(ot[:], psums[j, c][:],
                                         mybir.ActivationFunctionType.Copy)
                    nc.scalar.dma_start(out_d[ja, cs], ot[:])


_NC_CACHE = None
_LAST_IN_MAPS = None


def _get_nc():
    global _NC_CACHE
    if _NC_CACHE is None:
        _NC_CACHE = build_nc()
    return _NC_CACHE


def kernel(x, base_W, gate_W, lora_A, lora_B):
    x = np.asarray(x, dtype=np.float32)
    base_W = np.asarray(base_W, dtype=np.float32)
    gate_W = np.asarray(gate_W, dtype=np.float32)
    lora_A = np.asarray(lora_A, dtype=np.float32)
    lora_B = np.asarray(lora_B, dtype=np.float32)

    xf = x.reshape(B * S, D_IN)
    # W: [NG, P, KT, JG*P], wt[g,p,kt,c] = W[g*JG*P + c, kt*P + p]
    wpad = np.zeros((NG * JG * P, D_IN), dtype=BF)
    wpad[:D_OUT] = base_W.astype(BF)
    wt_np = np.ascontiguousarray(
        wpad.reshape(NG, JG * P, KT, P).transpose(0, 3, 2, 1))
    # at: [lora_A flat (224) | gate_W (28) | pad(4)] -> [P, KT*AT_COLS]
    M = np.zeros((AT_COLS, D_IN), np.float32)
    M[:ER] = lora_A.reshape(ER, D_IN)
    M[ER:ER + NE] = gate_W
    at_np = np.ascontiguousarray(
        M.T.reshape(KT, P, AT_COLS).transpose(1, 0, 2).reshape(
            P, KT * AT_COLS)).astype(BF)
    # lora_B -> b_flat [(e r), D_out], halves at er=128
    b_flat = lora_B.transpose(0, 2, 1).reshape(ER, D_OUT)

    ba_np = np.ascontiguousarray(b_flat[:P]).astype(BF)
    bb_np = np.ascontiguousarray(b_flat[P:]).astype(BF)

    in_maps = []
    for c in range(N_CORES):
        xc = xf[c * T:(c + 1) * T]                       # [T, D_in]
        xt_np = np.ascontiguousarray(
            xc.T.reshape(KT, P, T).transpose(1, 0, 2).reshape(
                P, KT * T)).astype(BF)
        in_maps.append({
            "xt": xt_np,
            "wt": wt_np,
            "at": at_np,
            "ba": ba_np,
            "bb": bb_np,
        })

    global _LAST_IN_MAPS
    _LAST_IN_MAPS = in_maps
    nc = _get_nc()
    res = bass_utils.run_bass_kernel_spmd(nc, in_maps,
                                          core_ids=list(range(N_CORES)))
    out = np.empty((B * S, D_OUT), dtype=np.float32)
    for c in range(N_CORES):
        out[c * T:(c + 1) * T] = res.results[c]["out"].T
    return out.reshape(B, S, D_OUT)
